# revision 18
# baseline (speedup 1.0000x reference)
"""Trainium2 Bass kernel for nn_HFMelSpectrogram (fp8 DoubleRow version).

Pipeline (per core, 4 batches of the 32-batch waveform):
  1. STFT-as-GEMM in fp8e4m3 with perf_mode=DoubleRow: the PE array is
     virtualized to 128x256 (2 fp8 weights per cell), so each matmul
     contracts 256 of the 1024 DFT samples at 2 MACs/cell/cycle.  Host
     packs the 1024 nontrivial DFT rows (513 cos + 511 sin) as
     pair-interleaved weight tiles [128p, 2s, 128j] and the frame matrix
     as [128p, 4ck, 2s, 1024t] with sample index n = 256*ck + 128*s + p.
     Scales (x*8, W*32) keep fp8e4m3 (max 240) well fed.
  2. Square (PSUM -> SBUF fp8, input scale 2^-11 keeps spec^2/64 <= ~222
     under the fp8 max), split per t-half: ScalarE squares the low bank
     while VectorE does the high bank (scale-to-bf16 then multiply; the
     DVE cannot dual-read PSUM).
  3. Mel projection, also fp8 DoubleRow: stationary mexp pairs
     [128, 2s, 64m] (i-block pairs), moving sq pairs -> psum [64m, 512t].
     The cos^2+sin^2 power sum is absorbed into the GEMM contraction.
     Issued as q0-q2 after m=6 and q3 + Ln + transpose after the batch,
     so the accumulation overlaps the next batch's STFT.
  4. Ln on ScalarE (scale 1/16 undoes the fp8 scaling; 10/log10 is folded
     into the resize matrix) -> logmel fp16 [64m, 1024t].
  5. Transpose logmel to [128t, ...] via the DMA XBAR (dma_start_transpose,
     one [64, 512] op per half-batch, zero engine cost), then bilinear
     height-resize 1000 -> 1024 as a banded fp16 GEMM.  The resize runs in
     two batch-pair halves: b0/b1 mid-run (hidden under b2/b3 STFT) and
     b2/b3 in the tail.
"""

import numpy as np
import ml_dtypes

import concourse.bass as bass
import concourse.bacc as bacc
import concourse.tile as tile
import concourse.mybir as mybir
from concourse.bass_utils import run_bass_kernel_spmd

F32 = mybir.dt.float32
F16 = mybir.dt.float16
BF16 = mybir.dt.bfloat16
F8 = mybir.dt.float8e4
E4 = ml_dtypes.float8_e4m3
DR = mybir.MatmulPerfMode.DoubleRow

N_FFT = 1024
HOP = 480
NB_MAX = 1000      # frames kept by the reference
N_MELS = 64
SPECW = 1024       # output height after resize
NBINS = 513
B, L = 32, 480000
NCORES = 8
BPC = B // NCORES  # batches per core
TFR = 1024         # padded frame count (frames >= 1000 are zeroed via R)
PAD = N_FFT // 2

SX = 8.0           # waveform fp8 scale
SW = 32.0          # DFT weight fp8 scale
SQS = 2.0 ** -11   # pre-square scale: sq = (spec*SQS)^2 = spec_true^2/64
SM = 1024.0        # mel filter fp8 scale
# mel_psum = (SX*SW*SQS)^2 * SM * mel_true = 16 * mel_true
LN_SCALE = 1.0 / 16.0

# Set by test harness to collect a profile; harness default leaves it off.
TRACE = False
LAST_RESULTS = None


def _resize_blocks():
    """Banded resize matrix blocks, f32 coords matching the reference."""
    scale = np.float32((NB_MAX - 1) / (SPECW - 1))
    pos = np.arange(SPECW, dtype=np.float32) * scale
    h0 = np.clip(np.floor(pos).astype(np.int64), 0, NB_MAX - 1)
    frac = (pos - h0.astype(np.float32)).astype(np.float64)
    h1 = np.minimum(h0 + 1, NB_MAX - 1)
    c = 10.0 / np.log(10.0)
    r = np.zeros((TFR, SPECW), np.float64)
    cols = np.arange(SPECW)
    r[h0, cols] += (1.0 - frac) * c
    r[h1, cols] += frac * c
    blocks = np.zeros((8, 2, 128, 128), np.float16)
    qpairs = []
    for g in range(8):
        sub = r[:, g * 128:(g + 1) * 128]
        rows = np.nonzero(sub.any(axis=1))[0]
        qs = sorted({int(q) for q in rows // 128})
        assert 1 <= len(qs) <= 2, qs
        q0 = qs[0]
        q1 = qs[1] if len(qs) > 1 else min(q0 + 1, 7)
        blocks[g, 0] = sub[q0 * 128:(q0 + 1) * 128].astype(np.float16)
        blocks[g, 1] = sub[q1 * 128:(q1 + 1) * 128].astype(np.float16)
        qpairs.append((q0, q1))
    return blocks, qpairs


_RBLOCKS, _QPAIRS = _resize_blocks()


def _build_bass():
    nc = bacc.Bacc("TRN2", target_bir_lowering=False, debug=False,
                   num_devices=NCORES)
    xt = nc.declare_dram_parameter("xt", [BPC, 128, 4, 2, TFR], F8,
                                   isOutput=False)
    wt = nc.declare_dram_parameter("wt", [8, 128, 4, 2, 128], F8,
                                   isOutput=False)
    mexp = nc.declare_dram_parameter("mexp", [4, 128, 2, N_MELS], F8,
                                     isOutput=False)
    rblk = nc.declare_dram_parameter("rblk", [8, 2, 128, 128], F16,
                                     isOutput=False)
    out = nc.declare_dram_parameter("out", [BPC, SPECW, N_MELS], F32,
                                    isOutput=True)
    MULT = mybir.AluOpType.mult

    with tile.TileContext(nc) as tc:
        with (
            tc.tile_pool(name="consts", bufs=1) as consts,
            tc.tile_pool(name="xt", bufs=1) as xpool,
            tc.tile_pool(name="sq", bufs=2) as sqpool,
            tc.tile_pool(name="tmp", bufs=2) as tmppool,
            tc.tile_pool(name="lm", bufs=1) as lmpool,
            tc.tile_pool(name="lmT", bufs=1) as lmTpool,
            tc.tile_pool(name="ot", bufs=2) as otpool,
            tc.tile_pool(name="specp", bufs=3, space="PSUM") as specp,
            tc.tile_pool(name="melp", bufs=2, space="PSUM") as melp,
        ):
            # warmup fodder (never read back): keeps the PE HAM window busy
            # while the first input DMAs land, so real matmuls start warm.
            # memset on GpSimd: its queue drains its preamble first (~6us),
            # 1.5us before the Vector queue would get there.
            wu = consts.tile([128, 2, 512], F8, tag="wu", name="wu")
            nc.gpsimd.memset(wu, 0.0)
            eps_t = consts.tile([128, 1], F32, tag="eps", name="eps")
            nc.vector.memset(eps_t, 1e-10)
            # activation-table preload (Square then Ln) during the head DMAs
            dmy = consts.tile([1, 1], F32, tag="dmy", name="dmy")
            nc.scalar.activation(out=dmy, in_=eps_t[:1, :],
                                 func=mybir.ActivationFunctionType.Square)
            nc.scalar.activation(out=dmy, in_=eps_t[:1, :],
                                 func=mybir.ActivationFunctionType.Ln,
                                 bias=eps_t[:1, :])

            # wt[0] gates the first real matmul: put it on the sync queue
            # (ready earliest) instead of behind the gpsimd const trickle.
            wt_t = []
            for m in range(8):
                t = consts.tile([128, 4, 2, 128], F8, tag=f"wt{m}",
                                name=f"wt{m}")
                (nc.sync if m == 0 else nc.gpsimd).dma_start(out=t, in_=wt[m])
                wt_t.append(t)
            mexp_t = []
            for q in range(4):
                t = consts.tile([128, 2, N_MELS], F8, tag=f"me{q}",
                                name=f"me{q}")
                nc.gpsimd.dma_start(out=t, in_=mexp[q])
                mexp_t.append(t)
            rb_t = []
            for g in range(8):
                pair = []
                for j in range(2):
                    t = consts.tile([128, 128], F16, tag=f"rb{g}_{j}",
                                    name=f"rb{g}_{j}")
                    nc.gpsimd.dma_start(out=t, in_=rblk[g, j])
                    pair.append(t)
                rb_t.append(pair)

            xt_t = []
            for b in range(BPC):
                t = xpool.tile([128, 4, 2, TFR], F8, tag=f"xt{b}",
                               name=f"xt{b}")
                # later batches load via the scalar HWDGE queue so xt0/xt1
                # are not stuck behind 2 MB of issue+transfer on sync
                (nc.sync if b < 2 else nc.scalar).dma_start(out=t, in_=xt[b])
                xt_t.append(t)

            sq_t = [[None] * 4 for _ in range(BPC)]
            lm_t = []
            for b in range(BPC):
                lm_t.append(lmpool.tile([N_MELS, TFR], F16, tag=f"lm{b}",
                                        name=f"lm{b}"))
            # transposed logmel: [t_local, t_chunk q, batch, mel]
            lmT = lmTpool.tile([128, 8, BPC, N_MELS], F16, tag="lmT",
                               name="lmT")

            # PE warmup: dummy matmuls racing the first input DMA.
            wups = specp.tile([128, TFR], F32, tag="spec", name="wups")
            for i in range(12):
                nc.tensor.matmul(wups[:, :512], lhsT=wu[:, :, :128], rhs=wu,
                                 start=True, stop=True, perf_mode=DR,
                                 skip_group_check=True)

            mel_ps = [None] * BPC

            def issue_mel_chunk1(b):
                """q0-q2 contributions for both t-halves."""
                mel_ps[b] = [melp.tile([N_MELS, 512], F32, tag="mel",
                                       name=f"mel{b}_{tc_}")
                             for tc_ in range(2)]
                for tc_ in range(2):
                    for q in range(3):
                        nc.tensor.matmul(
                            mel_ps[b][tc_],
                            lhsT=mexp_t[q],
                            rhs=sq_t[b][q][:, :, tc_ * 512:(tc_ + 1) * 512],
                            start=(q == 0),
                            stop=False,
                            perf_mode=DR,
                            skip_group_check=True,
                        )

            def issue_mel_chunk2(b):
                """q3 + Ln + transpose, per t-half."""
                for tc_ in range(2):
                    nc.tensor.matmul(
                        mel_ps[b][tc_],
                        lhsT=mexp_t[3],
                        rhs=sq_t[b][3][:, :, tc_ * 512:(tc_ + 1) * 512],
                        start=False,
                        stop=True,
                        perf_mode=DR,
                        skip_group_check=True,
                    )
                    nc.scalar.activation(
                        out=lm_t[b][:, tc_ * 512:(tc_ + 1) * 512],
                        in_=mel_ps[b][tc_],
                        func=mybir.ActivationFunctionType.Ln,
                        bias=eps_t[:N_MELS, :],
                        scale=LN_SCALE,
                    )
                    nc.sync.dma_start_transpose(
                        out=lmT[:, tc_ * 4:(tc_ + 1) * 4, b, :],
                        in_=lm_t[b][:, tc_ * 512:(tc_ + 1) * 512],
                    )

            def issue_resize(half):
                """Resize one batch pair; single 2-bank psum tile.
                Copy + store pipelined per 128-row block so the kernel
                tail is not one serial copy + giant DMA."""
                lo = half * 2
                rp = specp.tile([128, TFR], F32, tag="spec",
                                name=f"res{half}")
                ot = otpool.tile([128, 8, 2, N_MELS], F32, tag="ot",
                                 name=f"ot{half}")
                for g in range(8):
                    q0, q1 = _QPAIRS[g]
                    sl = rp[:, g * 128:(g + 1) * 128]
                    nc.tensor.matmul(sl, lhsT=rb_t[g][0],
                                     rhs=lmT[:, q0, lo:lo + 2, :],
                                     start=True, stop=False,
                                     skip_group_check=True)
                    nc.tensor.matmul(sl, lhsT=rb_t[g][1],
                                     rhs=lmT[:, q1, lo:lo + 2, :],
                                     start=False, stop=True,
                                     skip_group_check=True)
                # per-batch copy + store, parallel across engines/queues
                rp4 = rp.rearrange("p (g b m) -> p g b m", g=8, b=2)
                nc.scalar.copy(out=ot[:, :, 0, :], in_=rp4[:, :, 0, :])
                nc.vector.tensor_copy(out=ot[:, :, 1, :], in_=rp4[:, :, 1, :])
                nc.sync.dma_start(
                    out=out[lo, :, :].rearrange("(g h) m -> h g m", g=8),
                    in_=ot[:, :, 0, :],
                )
                nc.scalar.dma_start(
                    out=out[lo + 1, :, :].rearrange("(g h) m -> h g m", g=8),
                    in_=ot[:, :, 1, :],
                )

            pending = None
            for b in range(BPC):
                for m in range(8):
                    ps = specp.tile([128, TFR], F32, tag="spec",
                                    name="specpsum")
                    for ck in range(4):
                        for tc_ in range(2):
                            nc.tensor.matmul(
                                ps[:, tc_ * 512:(tc_ + 1) * 512],
                                lhsT=wt_t[m][:, ck],
                                rhs=xt_t[b][:, ck, :,
                                            tc_ * 512:(tc_ + 1) * 512],
                                start=(ck == 0),
                                stop=(ck == 3),
                                perf_mode=DR,
                                skip_group_check=True,
                            )
                    q, s = divmod(m, 2)
                    if s == 0:
                        sq_t[b][q] = sqpool.tile([128, 2, TFR], F8,
                                                 tag=f"sq{q}", name=f"sq{q}")
                    # square: ScalarE takes 3/4, VectorE the last 1/4
                    # (the DVE needs two passes, so it gets less work)
                    nc.scalar.activation(
                        out=sq_t[b][q][:, s, 0:768],
                        in_=ps[:, 0:768],
                        func=mybir.ActivationFunctionType.Square,
                        scale=SQS,
                    )
                    tmp = tmppool.tile([128, 256], BF16, tag="tmp",
                                       name="sqtmp")
                    nc.vector.tensor_scalar_mul(tmp, ps[:, 768:1024], SQS)
                    nc.vector.tensor_tensor(
                        out=sq_t[b][q][:, s, 768:1024],
                        in0=tmp, in1=tmp, op=MULT)
                    if m == 1 and pending is not None:
                        issue_mel_chunk2(pending)
                    if m == 4 and b == 2:
                        issue_resize(0)
                    if m == 6:
                        issue_mel_chunk1(b)
                pending = b
            issue_mel_chunk2(pending)
            issue_resize(1)
    return nc


def _host_prep(waveform, stft_weights, mel_filters):
    wv = np.ascontiguousarray(waveform, dtype=np.float32)
    xp = np.pad(wv, ((0, 0), (PAD, PAD)), mode="reflect")  # [B, 481024]
    need = HOP * (TFR - 1) + N_FFT  # 492064: max sample index + 1
    xz = np.zeros((B, need), np.float32)
    xz[:, : xp.shape[1]] = xp
    xq = (xz * SX).astype(E4)  # quantize once, then view strided
    sb = xq.strides[0]
    xt8 = np.lib.stride_tricks.as_strided(
        xq, shape=(B, 128, 4, 2, TFR), strides=(sb, 1, 256, 128, HOP))
    xt8 = np.ascontiguousarray(xt8)

    w = np.ascontiguousarray(stft_weights, dtype=np.float32)  # [1026, 1024]
    rows = list(range(0, NBINS)) + list(range(NBINS + 1, NBINS + 512))
    assert len(rows) == 1024
    wp = (w[rows] * SW).astype(E4)          # [1024 i, 1024 n]
    # i = 128*m + j ; n = 256*ck + 128*s + p  ->  [m, p, ck, s, j]
    wt8 = np.ascontiguousarray(
        wp.reshape(8, 128, 4, 2, 128).transpose(0, 4, 2, 3, 1))

    mf = np.ascontiguousarray(mel_filters, dtype=np.float32)  # [513, 64]
    f_of_i = np.array([i if i < NBINS else i - 512 for i in range(1024)])
    mexp = (mf[f_of_i] * SM).astype(E4)      # [1024 i, 64]
    # i = 128*(2q+s) + p -> [q, p, s, mm]
    mexp8 = np.ascontiguousarray(
        mexp.reshape(4, 2, 128, N_MELS).transpose(0, 2, 1, 3))
    return xt8, wt8, mexp8


def kernel(waveform, stft_weights, mel_filters):
    global LAST_RESULTS
    xt8, wt8, mexp8 = _host_prep(waveform, stft_weights, mel_filters)
    nc = _build_bass()
    in_maps = []
    for i in range(NCORES):
        in_maps.append({
            "xt": np.ascontiguousarray(xt8[i * BPC:(i + 1) * BPC]),
            "wt": wt8,
            "mexp": mexp8,
            "rblk": _RBLOCKS,
        })
    nc.compile()
    res = run_bass_kernel_spmd(nc, in_maps, list(range(NCORES)), trace=TRACE)
    LAST_RESULTS = res
    out = np.concatenate([r["out"] for r in res.results], axis=0)
    return out.reshape(B, 1, SPECW, N_MELS).astype(np.float32)


# revision 19
# speedup vs baseline: 1.0303x; 1.0303x over previous
"""Trainium2 Bass kernel for nn_HFMelSpectrogram (fp8 DoubleRow version).

Pipeline (per core, 4 batches of the 32-batch waveform):
  1. STFT-as-GEMM in fp8e4m3 with perf_mode=DoubleRow: the PE array is
     virtualized to 128x256 (2 fp8 weights per cell), so each matmul
     contracts 256 of the 1024 DFT samples at 2 MACs/cell/cycle.  Host
     packs the 1024 nontrivial DFT rows (513 cos + 511 sin) as
     pair-interleaved weight tiles [128p, 2s, 128j] and the frame matrix
     as [128p, 4ck, 2s, 1024t] with sample index n = 256*ck + 128*s + p.
     Scales (x*8, W*32) keep fp8e4m3 (max 240) well fed.
  2. Square (PSUM -> SBUF fp8, input scale 2^-11 keeps spec^2/64 <= ~222
     under the fp8 max), split per t-half: ScalarE squares the low bank
     while VectorE does the high bank (scale-to-bf16 then multiply; the
     DVE cannot dual-read PSUM).
  3. Mel projection, also fp8 DoubleRow: stationary mexp pairs
     [128, 2s, 64m] (i-block pairs), moving sq pairs -> psum [64m, 512t].
     The cos^2+sin^2 power sum is absorbed into the GEMM contraction.
     Issued as q0-q2 after m=6 and q3 + Ln + transpose after the batch,
     so the accumulation overlaps the next batch's STFT.
  4. Ln on ScalarE (scale 1/16 undoes the fp8 scaling; 10/log10 is folded
     into the resize matrix) -> logmel fp16 [64m, 1024t].
  5. Transpose logmel to [128t, ...] via the DMA XBAR (dma_start_transpose,
     one [64, 512] op per half-batch, zero engine cost), then bilinear
     height-resize 1000 -> 1024 as a banded fp16 GEMM.  The resize runs in
     two batch-pair halves: b0/b1 mid-run (hidden under b2/b3 STFT) and
     b2/b3 in the tail.
"""

import numpy as np
import ml_dtypes

import concourse.bass as bass
import concourse.bacc as bacc
import concourse.tile as tile
import concourse.mybir as mybir
from concourse.bass_utils import run_bass_kernel_spmd

F32 = mybir.dt.float32
F16 = mybir.dt.float16
BF16 = mybir.dt.bfloat16
F8 = mybir.dt.float8e4
E4 = ml_dtypes.float8_e4m3
DR = mybir.MatmulPerfMode.DoubleRow

N_FFT = 1024
HOP = 480
NB_MAX = 1000      # frames kept by the reference
N_MELS = 64
SPECW = 1024       # output height after resize
NBINS = 513
B, L = 32, 480000
NCORES = 8
BPC = B // NCORES  # batches per core
TFR = 1024         # padded frame count (frames >= 1000 are zeroed via R)
PAD = N_FFT // 2

SX = 8.0           # waveform fp8 scale
SW = 32.0          # DFT weight fp8 scale
SQS = 2.0 ** -11   # pre-square scale: sq = (spec*SQS)^2 = spec_true^2/64
SM = 1024.0        # mel filter fp8 scale
# mel_psum = (SX*SW*SQS)^2 * SM * mel_true = 16 * mel_true
LN_SCALE = 1.0 / 16.0

# Set by test harness to collect a profile; harness default leaves it off.
TRACE = False
LAST_RESULTS = None


def _resize_blocks():
    """Banded resize matrix blocks, f32 coords matching the reference."""
    scale = np.float32((NB_MAX - 1) / (SPECW - 1))
    pos = np.arange(SPECW, dtype=np.float32) * scale
    h0 = np.clip(np.floor(pos).astype(np.int64), 0, NB_MAX - 1)
    frac = (pos - h0.astype(np.float32)).astype(np.float64)
    h1 = np.minimum(h0 + 1, NB_MAX - 1)
    c = 10.0 / np.log(10.0)
    r = np.zeros((TFR, SPECW), np.float64)
    cols = np.arange(SPECW)
    r[h0, cols] += (1.0 - frac) * c
    r[h1, cols] += frac * c
    blocks = np.zeros((8, 2, 128, 128), np.float16)
    qpairs = []
    for g in range(8):
        sub = r[:, g * 128:(g + 1) * 128]
        rows = np.nonzero(sub.any(axis=1))[0]
        qs = sorted({int(q) for q in rows // 128})
        assert 1 <= len(qs) <= 2, qs
        q0 = qs[0]
        q1 = qs[1] if len(qs) > 1 else min(q0 + 1, 7)
        blocks[g, 0] = sub[q0 * 128:(q0 + 1) * 128].astype(np.float16)
        blocks[g, 1] = sub[q1 * 128:(q1 + 1) * 128].astype(np.float16)
        qpairs.append((q0, q1))
    return blocks, qpairs


_RBLOCKS, _QPAIRS = _resize_blocks()


def _build_bass():
    nc = bacc.Bacc("TRN2", target_bir_lowering=False, debug=False,
                   num_devices=NCORES)
    xt = nc.declare_dram_parameter("xt", [BPC, 128, 4, 2, TFR], F8,
                                   isOutput=False)
    wt = nc.declare_dram_parameter("wt", [8, 128, 4, 2, 128], F8,
                                   isOutput=False)
    mexp = nc.declare_dram_parameter("mexp", [4, 128, 2, N_MELS], F8,
                                     isOutput=False)
    rblk = nc.declare_dram_parameter("rblk", [8, 2, 128, 128], F16,
                                     isOutput=False)
    out = nc.declare_dram_parameter("out", [BPC, SPECW, N_MELS], F32,
                                    isOutput=True)
    MULT = mybir.AluOpType.mult

    with tile.TileContext(nc) as tc:
        with (
            tc.tile_pool(name="consts", bufs=1) as consts,
            tc.tile_pool(name="xt", bufs=1) as xpool,
            tc.tile_pool(name="sq", bufs=2) as sqpool,
            tc.tile_pool(name="tmp", bufs=2) as tmppool,
            tc.tile_pool(name="lm", bufs=1) as lmpool,
            tc.tile_pool(name="lmT", bufs=1) as lmTpool,
            tc.tile_pool(name="ot", bufs=2) as otpool,
            tc.tile_pool(name="specp", bufs=3, space="PSUM") as specp,
            tc.tile_pool(name="melp", bufs=2, space="PSUM") as melp,
        ):
            # warmup fodder (never read back): keeps the PE HAM window busy
            # while the first input DMAs land, so real matmuls start warm.
            # memset on GpSimd: its queue drains its preamble first (~6us),
            # 1.5us before the Vector queue would get there.
            wu = consts.tile([128, 2, 512], F8, tag="wu", name="wu")
            nc.gpsimd.memset(wu, 0.0)
            eps_t = consts.tile([128, 1], F32, tag="eps", name="eps")
            nc.vector.memset(eps_t, 1e-10)
            # activation-table preload (Square then Ln) during the head DMAs
            dmy = consts.tile([1, 1], F32, tag="dmy", name="dmy")
            nc.scalar.activation(out=dmy, in_=eps_t[:1, :],
                                 func=mybir.ActivationFunctionType.Square)
            nc.scalar.activation(out=dmy, in_=eps_t[:1, :],
                                 func=mybir.ActivationFunctionType.Ln,
                                 bias=eps_t[:1, :])

            # wt[0] gates the first real matmul: put it on the sync queue
            # (ready earliest) instead of behind the gpsimd const trickle.
            wt_t = []
            for m in range(8):
                t = consts.tile([128, 4, 2, 128], F8, tag=f"wt{m}",
                                name=f"wt{m}")
                (nc.sync if m == 0 else nc.gpsimd).dma_start(out=t, in_=wt[m])
                wt_t.append(t)
            mexp_t = []
            for q in range(4):
                t = consts.tile([128, 2, N_MELS], F8, tag=f"me{q}",
                                name=f"me{q}")
                nc.gpsimd.dma_start(out=t, in_=mexp[q])
                mexp_t.append(t)
            rb_t = []
            for g in range(8):
                pair = []
                for j in range(2):
                    t = consts.tile([128, 128], F16, tag=f"rb{g}_{j}",
                                    name=f"rb{g}_{j}")
                    nc.gpsimd.dma_start(out=t, in_=rblk[g, j])
                    pair.append(t)
                rb_t.append(pair)

            xt_t = []
            for b in range(BPC):
                t = xpool.tile([128, 4, 2, TFR], F8, tag=f"xt{b}",
                               name=f"xt{b}")
                nc.sync.dma_start(out=t, in_=xt[b])
                xt_t.append(t)

            sq_t = [[None] * 4 for _ in range(BPC)]
            lm_t = []
            for b in range(BPC):
                lm_t.append(lmpool.tile([N_MELS, TFR], F16, tag=f"lm{b}",
                                        name=f"lm{b}"))
            # transposed logmel: [t_local, t_chunk q, batch, mel]
            lmT = lmTpool.tile([128, 8, BPC, N_MELS], F16, tag="lmT",
                               name="lmT")

            # PE warmup: dummy matmuls racing the first input DMA.
            wups = specp.tile([128, TFR], F32, tag="spec", name="wups")
            for i in range(12):
                nc.tensor.matmul(wups[:, :512], lhsT=wu[:, :, :128], rhs=wu,
                                 start=True, stop=True, perf_mode=DR,
                                 skip_group_check=True)

            mel_ps = [None] * BPC

            def issue_mel_chunk1(b):
                """q0-q2 contributions for both t-halves."""
                mel_ps[b] = [melp.tile([N_MELS, 512], F32, tag="mel",
                                       name=f"mel{b}_{tc_}")
                             for tc_ in range(2)]
                for tc_ in range(2):
                    for q in range(3):
                        nc.tensor.matmul(
                            mel_ps[b][tc_],
                            lhsT=mexp_t[q],
                            rhs=sq_t[b][q][:, :, tc_ * 512:(tc_ + 1) * 512],
                            start=(q == 0),
                            stop=False,
                            perf_mode=DR,
                            skip_group_check=True,
                        )

            def issue_mel_chunk2(b):
                """q3 + Ln + transpose, per t-half."""
                for tc_ in range(2):
                    nc.tensor.matmul(
                        mel_ps[b][tc_],
                        lhsT=mexp_t[3],
                        rhs=sq_t[b][3][:, :, tc_ * 512:(tc_ + 1) * 512],
                        start=False,
                        stop=True,
                        perf_mode=DR,
                        skip_group_check=True,
                    )
                    nc.scalar.activation(
                        out=lm_t[b][:, tc_ * 512:(tc_ + 1) * 512],
                        in_=mel_ps[b][tc_],
                        func=mybir.ActivationFunctionType.Ln,
                        bias=eps_t[:N_MELS, :],
                        scale=LN_SCALE,
                    )
                    nc.sync.dma_start_transpose(
                        out=lmT[:, tc_ * 4:(tc_ + 1) * 4, b, :],
                        in_=lm_t[b][:, tc_ * 512:(tc_ + 1) * 512],
                    )

            def issue_resize(half):
                """Resize one batch pair; single 2-bank psum tile.
                Copy + store pipelined per 128-row block so the kernel
                tail is not one serial copy + giant DMA."""
                lo = half * 2
                rp = specp.tile([128, TFR], F32, tag="spec",
                                name=f"res{half}")
                ot = otpool.tile([128, 8, 2, N_MELS], F32, tag="ot",
                                 name=f"ot{half}")
                for g in range(8):
                    q0, q1 = _QPAIRS[g]
                    sl = rp[:, g * 128:(g + 1) * 128]
                    nc.tensor.matmul(sl, lhsT=rb_t[g][0],
                                     rhs=lmT[:, q0, lo:lo + 2, :],
                                     start=True, stop=False,
                                     skip_group_check=True)
                    nc.tensor.matmul(sl, lhsT=rb_t[g][1],
                                     rhs=lmT[:, q1, lo:lo + 2, :],
                                     start=False, stop=True,
                                     skip_group_check=True)
                # per-batch copy + store, parallel across engines/queues
                rp4 = rp.rearrange("p (g b m) -> p g b m", g=8, b=2)
                nc.scalar.copy(out=ot[:, :, 0, :], in_=rp4[:, :, 0, :])
                nc.vector.tensor_copy(out=ot[:, :, 1, :], in_=rp4[:, :, 1, :])
                nc.sync.dma_start(
                    out=out[lo, :, :].rearrange("(g h) m -> h g m", g=8),
                    in_=ot[:, :, 0, :],
                )
                nc.scalar.dma_start(
                    out=out[lo + 1, :, :].rearrange("(g h) m -> h g m", g=8),
                    in_=ot[:, :, 1, :],
                )

            pending = None
            for b in range(BPC):
                for m in range(8):
                    ps = specp.tile([128, TFR], F32, tag="spec",
                                    name="specpsum")
                    for ck in range(4):
                        for tc_ in range(2):
                            nc.tensor.matmul(
                                ps[:, tc_ * 512:(tc_ + 1) * 512],
                                lhsT=wt_t[m][:, ck],
                                rhs=xt_t[b][:, ck, :,
                                            tc_ * 512:(tc_ + 1) * 512],
                                start=(ck == 0),
                                stop=(ck == 3),
                                perf_mode=DR,
                                skip_group_check=True,
                            )
                    q, s = divmod(m, 2)
                    if s == 0:
                        sq_t[b][q] = sqpool.tile([128, 2, TFR], F8,
                                                 tag=f"sq{q}", name=f"sq{q}")
                    # square: ScalarE takes 3/4, VectorE the last 1/4
                    # (the DVE needs two passes, so it gets less work)
                    nc.scalar.activation(
                        out=sq_t[b][q][:, s, 0:768],
                        in_=ps[:, 0:768],
                        func=mybir.ActivationFunctionType.Square,
                        scale=SQS,
                    )
                    tmp = tmppool.tile([128, 256], BF16, tag="tmp",
                                       name="sqtmp")
                    nc.vector.tensor_scalar_mul(tmp, ps[:, 768:1024], SQS)
                    nc.vector.tensor_tensor(
                        out=sq_t[b][q][:, s, 768:1024],
                        in0=tmp, in1=tmp, op=MULT)
                    if m == 1 and pending is not None:
                        issue_mel_chunk2(pending)
                    if m == 4 and b == 2:
                        issue_resize(0)
                    if m == 6:
                        issue_mel_chunk1(b)
                pending = b
            issue_mel_chunk2(pending)
            issue_resize(1)
    return nc


def _host_prep(waveform, stft_weights, mel_filters):
    wv = np.ascontiguousarray(waveform, dtype=np.float32)
    xp = np.pad(wv, ((0, 0), (PAD, PAD)), mode="reflect")  # [B, 481024]
    need = HOP * (TFR - 1) + N_FFT  # 492064: max sample index + 1
    xz = np.zeros((B, need), np.float32)
    xz[:, : xp.shape[1]] = xp
    xq = (xz * SX).astype(E4)  # quantize once, then view strided
    sb = xq.strides[0]
    xt8 = np.lib.stride_tricks.as_strided(
        xq, shape=(B, 128, 4, 2, TFR), strides=(sb, 1, 256, 128, HOP))
    xt8 = np.ascontiguousarray(xt8)

    w = np.ascontiguousarray(stft_weights, dtype=np.float32)  # [1026, 1024]
    rows = list(range(0, NBINS)) + list(range(NBINS + 1, NBINS + 512))
    assert len(rows) == 1024
    wp = (w[rows] * SW).astype(E4)          # [1024 i, 1024 n]
    # i = 128*m + j ; n = 256*ck + 128*s + p  ->  [m, p, ck, s, j]
    wt8 = np.ascontiguousarray(
        wp.reshape(8, 128, 4, 2, 128).transpose(0, 4, 2, 3, 1))

    mf = np.ascontiguousarray(mel_filters, dtype=np.float32)  # [513, 64]
    f_of_i = np.array([i if i < NBINS else i - 512 for i in range(1024)])
    mexp = (mf[f_of_i] * SM).astype(E4)      # [1024 i, 64]
    # i = 128*(2q+s) + p -> [q, p, s, mm]
    mexp8 = np.ascontiguousarray(
        mexp.reshape(4, 2, 128, N_MELS).transpose(0, 2, 1, 3))
    return xt8, wt8, mexp8


def kernel(waveform, stft_weights, mel_filters):
    global LAST_RESULTS
    xt8, wt8, mexp8 = _host_prep(waveform, stft_weights, mel_filters)
    nc = _build_bass()
    in_maps = []
    for i in range(NCORES):
        in_maps.append({
            "xt": np.ascontiguousarray(xt8[i * BPC:(i + 1) * BPC]),
            "wt": wt8,
            "mexp": mexp8,
            "rblk": _RBLOCKS,
        })
    nc.compile()
    res = run_bass_kernel_spmd(nc, in_maps, list(range(NCORES)), trace=TRACE)
    LAST_RESULTS = res
    out = np.concatenate([r["out"] for r in res.results], axis=0)
    return out.reshape(B, 1, SPECW, N_MELS).astype(np.float32)
